# revision 25
# baseline (speedup 1.0000x reference)
# Trainium2 Bass kernel for single-head attention:
#   out = softmax((q@Wq+bq)(k@Wk+bk)^T / sqrt(D)) @ (v@Wv+bv) @ Wo + bo
# Full shapes: query/key/value [4, 2048, 1024], D=1024, mask all-ones.
#
# Algebraic folding (the big flop saver): softmax is invariant to adding a
# per-row constant, and its rows sum to one, so
#   scores ~ (q @ A + c) @ k^T        A = Wq Wk^T / sqrt(D),  c = bq Wk^T/sqrt(D)
#   out    = (P @ v) @ Wvo + bo_eff   Wvo = Wv Wo,  bo_eff = bv Wo + bo
# (the bk term only shifts each score row by a constant -> dropped; bv commutes
# through the normalized softmax average -> folded into bo_eff, added on host).
# A and Wvo are precomputed on the host in fp32 and cast to bf16. This removes
# the k and v projections entirely: per-core work drops from 8.6 to 6.4 GMAC
# (768 N=512 matmuls, 164 us PE floor), and no cross-core collective is needed
# - each core receives the full raw k/v of its batch as inputs.
#
# Sharding: data-parallel over (batch, query-half) -> 8 shards, one per
# NeuronCore. Core c handles batch b=c//2, query rows [h*1024,(h+1)*1024),
# h=c%2, against all 2048 k/v tokens of that batch.
#
# Layout: everything feature-major ("transposed"), zero on-chip transposes.
#   QT = (A^T qT)+c      [D, 1024]  dt-outer accumulation so the first matmul
#     only needs the first 256 KB of inputs (DMA chunks land just ahead of
#     consumption; all DMA chunks are partition-contiguous - descriptor
#     generation costs ~5ns per partition*segment, so segmented APs are
#     poison). Four (c-half, o-half) blocks alternate between the two PSUM
#     pools so evictions never stall the PE.
#   scoresT[k,q] over d: matmul(lhsT=KT k-tile, rhs=QT q-block); ACT Exp
#     evicts P^T[k,q] bf16 (no max-subtraction: |scores| <~ 6 here, exp is
#     safe in fp32). P stays UNNORMALIZED; 1/rowsum is applied as the
#     per-partition ACT scale of the output projection.
#   attn_outT[dv,q] = matmul(lhsT=V token-major tile, rhs=P^T) over 16
#     k-tiles, DVE-evicted (scalar engine carries zero DMAs and only ACTs,
#     so PSUM evictions never queue behind descriptor generation).
#   out[tok,dout] = matmul(lhsT=attn_outT tile, rhs=Wvo chunk)
# Row sums: incremental DVE adds over k-tiles of P^T, then a GpSimd partition
# reduce + DRAM-bounce partition scatter + reciprocal, all off the PE path.
# V/Wvo DMA issues are gated behind a dummy DVE copy that depends on the
# first q_eff block, so the early HBM bandwidth all goes to A/q/K.

import functools

import ml_dtypes
import numpy as np

B, S, D = 4, 2048, 1024
N_CORES = 8
P = 128
DT = D // P        # 8 d-tiles of 128
TQ = S // 2        # 1024 query rows per core
NQ = TQ // P       # 8 q-tiles
NK = S // P        # 16 k-tiles
SCALE = 1.0 / np.sqrt(np.float32(D))  # 1/32
BF16 = ml_dtypes.bfloat16


@functools.lru_cache(maxsize=1)
def _build():
    import concourse.bass as bass  # noqa: F401  (registers engines)
    import concourse.mybir as mybir
    import concourse.tile as tile
    from concourse import bacc

    f32 = mybir.dt.float32
    bf16 = mybir.dt.bfloat16

    nc = bacc.Bacc("TRN2", target_bir_lowering=False, debug=False,
                   num_devices=N_CORES)

    qT = nc.dram_tensor("qT", [D, TQ], bf16, kind="ExternalInput")
    kT = nc.dram_tensor("kT", [D, S], bf16, kind="ExternalInput")
    vtm = nc.dram_tensor("vtm", [S, D], bf16, kind="ExternalInput")
    wa = nc.dram_tensor("wa", [D, D], bf16, kind="ExternalInput")    # A
    wvo = nc.dram_tensor("wvo", [D, D], bf16, kind="ExternalInput")  # Wv@Wo
    bc = nc.dram_tensor("bc", [D], f32, kind="ExternalInput")  # bq@Wk^T/32
    out_d = nc.dram_tensor("out", [TQ, D], f32, kind="ExternalOutput")

    Ident = mybir.ActivationFunctionType.Identity
    Exp = mybir.ActivationFunctionType.Exp

    with tile.TileContext(nc) as tc:
        with (
            tc.tile_pool(name="const", bufs=1) as const,
            tc.tile_pool(name="wpool", bufs=2) as wpool,
            tc.tile_pool(name="big", bufs=1) as big,
            tc.tile_pool(name="work", bufs=2) as work,
            tc.tile_pool(name="sums", bufs=1) as sums,
            tc.tile_pool(name="dram", bufs=1, space="DRAM") as dram,
            tc.tile_pool(name="mmps", bufs=4, space="PSUM") as mmps,
            tc.tile_pool(name="scps", bufs=4, space="PSUM") as scps,
        ):
            # ---- constants ----
            bc_sb = const.tile([P, DT], f32, tag="bc")
            r_all = const.tile([P, NQ], f32, tag="rall")
            warm = const.tile([P, 640], bf16, tag="warm")

            # ---- persistent intermediates ----
            QT = big.tile([P, DT, TQ], bf16, tag="QT")       # 2 MB  q_eff^T
            KT = big.tile([P, DT, S], bf16, tag="KT")        # 4 MB
            Vtm = big.tile([P, NK, D], bf16, tag="Vtm")      # 4 MB (token-major)
            PT = big.tile([P, NK, TQ], bf16, tag="PT")       # 4 MB  exp(s)^T
            aoT = big.tile([P, DT, TQ], bf16, tag="aoT")     # 2 MB attn_out^T
            qsb = big.tile([P, DT, TQ], bf16, tag="qsb")     # 2 MB raw q^T
            w_a = wpool.tile([P, DT, D], bf16, tag="w", name="w_a")
            w_o = wpool.tile([P, DT, D], bf16, tag="w", name="w_o")

            # ---- PE warm-up: a dozen matmuls on a memset tile keep the
            # PE busy from ~6us (during the input DMA latency) so the HAM
            # clock-gate is already at 8/8 when the real stream starts ----
            nc.gpsimd.memset(warm[:], 0.001)
            warmps = mmps.tile([P, 512], f32, tag="mm", name="warmps")
            for _ in range(7):
                nc.tensor.matmul(warmps[:], warm[:, 512:640], warm[:, 0:512],
                                 start=True, stop=True)

            # ---- input DMAs: contiguous chunks, in consumption order ----
            wa_ap = wa.ap().rearrange("(dt p) n -> p dt n", p=P)
            q_ap = qT.ap().rearrange("(dt p) t -> p dt t", p=P)
            # a throwaway first DMA per queue absorbs the ~2us DMA-ring
            # spin-up latency so the first real chunk lands sooner
            spin = work.tile([P, 64], bf16, tag="spin")
            nc.gpsimd.dma_start(spin[:, 0:32], wa_ap[:, 0, 0:32])
            nc.sync.dma_start(spin[:, 32:64], q_ap[:, 0, 0:32])
            # gpsimd queue: the half of A the very first matmuls need, then
            # the (tiny, partition-major) bias, then the rest of A by
            # full-dt chunks (dt-outer q_eff consumes them in this order)
            nc.gpsimd.dma_start(w_a[:, 0, 0:512], wa_ap[:, 0, 0:512])
            nc.gpsimd.dma_start(bc_sb[:], bc.ap().rearrange("(p o) -> p o", p=P))
            nc.gpsimd.dma_start(w_a[:, 0, 512:1024], wa_ap[:, 0, 512:1024])
            for dt_i in range(1, DT):
                nc.gpsimd.dma_start(w_a[:, dt_i, :], wa_ap[:, dt_i, :])
            # sync queue: q by (c-half, dt) chunks
            for ch in range(2):
                for dt_i in range(DT):
                    sl = slice(ch * 512, (ch + 1) * 512)
                    nc.sync.dma_start(qsb[:, dt_i, sl], q_ap[:, dt_i, sl])
            # gpsimd continues with K (needed when scores start, ~40us in)
            k_ap = kT.ap().rearrange("(dt p) t -> p dt t", p=P)
            for dt_i in range(DT):
                nc.gpsimd.dma_start(KT[:, dt_i, :], k_ap[:, dt_i, :])

            # ---- q_eff: four (c, o-half) blocks, dt-outer accumulation ----
            # QT[:, o, csl] = sum_dt A[:, dt, oP:+P]^T qsb[:, dt, csl] + bc
            def qeff_block(ch, oh, pool, tg):
                csl = slice(ch * 512, (ch + 1) * 512)
                pss = [pool.tile([P, 512], f32, tag=tg,
                                 name=f"qe{ch}{oh}{i}") for i in range(4)]
                for dt_i in range(DT):
                    for oi in range(4):
                        o = oh * 4 + oi
                        nc.tensor.matmul(
                            pss[oi][:],
                            w_a[:, dt_i, o * P:(o + 1) * P],
                            qsb[:, dt_i, csl],
                            start=(dt_i == 0),
                            stop=(dt_i == DT - 1),
                        )
                for oi in range(4):
                    o = oh * 4 + oi
                    nc.scalar.activation(
                        QT[:, o, csl], pss[oi][:],
                        Ident, bias=bc_sb[:, o:o + 1], scale=1.0,
                    )

            qeff_block(0, 0, mmps, "mm")
            qeff_block(0, 1, scps, "sc")
            qeff_block(1, 0, mmps, "mm")
            qeff_block(1, 1, scps, "sc")

            # gpsimd tail: V and Wvo loads, gated on the first q_eff block
            # having evicted, so they don't steal early HBM bandwidth
            gate = work.tile([P, 1], bf16, tag="gate")
            nc.gpsimd.tensor_copy(gate[:], QT[:, 3, 0:1])
            v_ap = vtm.ap().rearrange("(kt p) d -> p kt d", p=P)
            for c in range(4):
                nc.gpsimd.dma_start(Vtm[:, c * 4:(c + 1) * 4, :],
                                    v_ap[:, c * 4:(c + 1) * 4, :])
            wo_ap = wvo.ap().rearrange("(dt p) n -> p dt n", p=P)
            for dt_i in range(DT):
                nc.gpsimd.dma_start(w_o[:, dt_i, :], wo_ap[:, dt_i, :])

            # ---- attention over 512-wide q-blocks ----
            s1_tiles = {}

            def s1_add(blk, kt):
                # incremental k-tile sum on the DVE (contiguous reads)
                qsl = slice(blk * 512, (blk + 1) * 512)
                s1 = s1_tiles[blk]
                if kt == 0:
                    nc.vector.tensor_copy(s1[:], PT[:, 0, qsl])
                else:
                    nc.vector.tensor_tensor(
                        s1[:], PT[:, kt, qsl], s1[:], mybir.AluOpType.add)

            def score_blk(blk):
                qsl = slice(blk * 512, (blk + 1) * 512)
                s1_tiles[blk] = sums.tile([P, 512], f32, tag=f"s1b{blk}",
                                          name=f"s1b{blk}")
                for kt in range(NK):
                    sc = scps.tile([P, 512], f32, tag="sc")
                    for dt_i in range(DT):
                        nc.tensor.matmul(
                            sc[:],
                            KT[:, dt_i, kt * P:(kt + 1) * P],
                            QT[:, dt_i, qsl],
                            start=(dt_i == 0),
                            stop=(dt_i == DT - 1),
                        )
                    nc.scalar.activation(PT[:, kt, qsl], sc[:], Exp)
                    s1_add(blk, kt)

            def row_sums(blk):
                # 1/rowsum for the block's 512 q positions -> r_all[:, 4b:4b+4]
                s1 = s1_tiles[blk]
                from concourse import bass_isa
                s2 = sums.tile([P, 512], f32, tag=f"s2b{blk}", name=f"s2b{blk}")
                nc.gpsimd.partition_all_reduce(
                    s2[:], s1[:], channels=P, reduce_op=bass_isa.ReduceOp.add)
                rsc = work.tile([P, 4], f32, tag="rsc")
                # partition-scatter must bounce through DRAM: SBUF partition
                # dims are physical and cannot be built from free strides
                rd = dram.tile([1, 512], f32, name=f"rd{blk}")
                nc.sync.dma_start(rd[:], s2[0:1, :])
                nc.sync.dma_start(
                    rsc[:], rd[:].rearrange("a (j p) -> p (a j)", p=P))
                nc.vector.reciprocal(r_all[:, blk * 4:(blk + 1) * 4], rsc[:])

            def attn_v(blk):
                # aoT[:, dvt, qsl] = sum_kt Vtm[:, kt, dvt*P:+P]^T P^T[:, kt, qsl]
                qsl = slice(blk * 512, (blk + 1) * 512)
                for dvt in range(DT):
                    av = mmps.tile([P, 512], f32, tag="mm")
                    for kt in range(NK):
                        nc.tensor.matmul(
                            av[:],
                            Vtm[:, kt, dvt * P:(dvt + 1) * P],
                            PT[:, kt, qsl],
                            start=(kt == 0),
                            stop=(kt == NK - 1),
                        )
                    nc.vector.tensor_copy(aoT[:, dvt, qsl], av[:])

            def out_proj(tt, width=512):
                # out[tok, dout], scaled by 1/rowsum (tokens on partitions).
                # The very last tile runs at width 256 so its trailing
                # ACT+DMA chain (the kernel's tail) is half as long.
                fin = work.tile([P, D], f32, tag="fin")
                for dc in range(D // width):
                    osl = slice(dc * width, (dc + 1) * width)
                    ps = scps.tile([P, 512], f32, tag="sc")
                    for dvt in range(DT):
                        nc.tensor.matmul(
                            ps[:, 0:width],
                            aoT[:, dvt, tt * P:(tt + 1) * P],
                            w_o[:, dvt, osl],
                            start=(dvt == 0),
                            stop=(dvt == DT - 1),
                        )
                    nc.scalar.activation(
                        fin[:, osl], ps[:, 0:width],
                        Ident, scale=r_all[:, tt:tt + 1],
                    )
                    nc.sync.dma_start(
                        out_d.ap()[tt * P:(tt + 1) * P, osl], fin[:, osl])

            score_blk(0)
            attn_v(0)
            row_sums(0)
            score_blk(1)
            attn_v(1)
            row_sums(1)
            for tt in range(NQ):
                out_proj(tt, width=(256 if tt == NQ - 1 else 512))

    nc.compile()
    return nc


def _numpy_reference(query, key, value, mask, Wq, bq, Wk, bk, Wv, bv, Wo, bo):
    q = query @ Wq + bq
    k = key @ Wk + bk
    v = value @ Wv + bv
    s = np.einsum("bsd,btd->bst", q, k) / np.sqrt(np.float32(q.shape[-1]))
    s = np.where(mask == 0, np.float32(-1e9), s)
    s = s - s.max(axis=-1, keepdims=True)
    e = np.exp(s)
    p = e / e.sum(axis=-1, keepdims=True)
    x = np.einsum("bst,btd->bsd", p, v)
    return (x @ Wo + bo).astype(np.float32)


def kernel(query, key, value, mask, Wq, bq, Wk, bk, Wv, bv, Wo, bo):
    query = np.asarray(query, np.float32)
    key = np.asarray(key, np.float32)
    value = np.asarray(value, np.float32)
    mask = np.asarray(mask)
    if not np.all(mask != 0):
        # This problem's mask is always all-ones; keep a correct fallback.
        return _numpy_reference(query, key, value, mask, Wq, bq, Wk, bk,
                                Wv, bv, Wo, bo)

    from concourse.bass_utils import run_bass_kernel_spmd

    nc = _build()

    Wq32 = np.asarray(Wq, np.float32)
    Wk32 = np.asarray(Wk, np.float32)
    Wv32 = np.asarray(Wv, np.float32)
    Wo32 = np.asarray(Wo, np.float32)
    A = (Wq32 @ Wk32.T * SCALE).astype(BF16)          # scores = (q A + c) k^T
    Wvo = (Wv32 @ Wo32).astype(BF16)
    c_vec = (np.asarray(bq, np.float32) @ Wk32.T * SCALE).astype(np.float32)
    # pack partition-major: kernel reads bc_sb[p, o] = c_vec[o*P + p]
    c_vec = np.ascontiguousarray(c_vec.reshape(DT, P).T).ravel()
    bo_eff = (np.asarray(bv, np.float32) @ Wo32
              + np.asarray(bo, np.float32)).astype(np.float32)

    kT_b = [np.ascontiguousarray(key[b].T).astype(BF16) for b in range(B)]
    v_b = [np.ascontiguousarray(value[b]).astype(BF16) for b in range(B)]

    in_maps = []
    for cidx in range(N_CORES):
        b, h = divmod(cidx, 2)
        sl = slice(h * TQ, (h + 1) * TQ)
        in_maps.append({
            "qT": np.ascontiguousarray(query[b, sl].T).astype(BF16),
            "kT": kT_b[b],
            "vtm": v_b[b],
            "wa": A, "wvo": Wvo, "bc": c_vec,
        })

    global _last_in_maps
    _last_in_maps = in_maps
    res = run_bass_kernel_spmd(nc, in_maps, list(range(N_CORES)))

    out = np.empty((B, S, D), np.float32)
    for cidx in range(N_CORES):
        b, h = divmod(cidx, 2)
        out[b, h * TQ:(h + 1) * TQ] = res.results[cidx]["out"]
    out += bo_eff
    return out


# revision 26
# speedup vs baseline: 1.0197x; 1.0197x over previous
# Trainium2 Bass kernel for single-head attention:
#   out = softmax((q@Wq+bq)(k@Wk+bk)^T / sqrt(D)) @ (v@Wv+bv) @ Wo + bo
# Full shapes: query/key/value [4, 2048, 1024], D=1024, mask all-ones.
#
# Algebraic folding (the big flop saver): softmax is invariant to adding a
# per-row constant, and its rows sum to one, so
#   scores ~ (q @ A + c) @ k^T        A = Wq Wk^T / sqrt(D),  c = bq Wk^T/sqrt(D)
#   out    = (P @ v) @ Wvo + bo_eff   Wvo = Wv Wo,  bo_eff = bv Wo + bo
# (the bk term only shifts each score row by a constant -> dropped; bv commutes
# through the normalized softmax average -> folded into bo_eff, added on host).
# A and Wvo are precomputed on the host in fp32 and cast to bf16. This removes
# the k and v projections entirely: per-core work drops from 8.6 to 6.4 GMAC
# (768 N=512 matmuls, 164 us PE floor), and no cross-core collective is needed
# - each core receives the full raw k/v of its batch as inputs.
#
# Sharding: data-parallel over (batch, query-half) -> 8 shards, one per
# NeuronCore. Core c handles batch b=c//2, query rows [h*1024,(h+1)*1024),
# h=c%2, against all 2048 k/v tokens of that batch.
#
# Layout: everything feature-major ("transposed"), zero on-chip transposes.
#   QT = (A^T qT)+c      [D, 1024]  dt-outer accumulation so the first matmul
#     only needs the first 256 KB of inputs (DMA chunks land just ahead of
#     consumption; all DMA chunks are partition-contiguous - descriptor
#     generation costs ~5ns per partition*segment, so segmented APs are
#     poison). Four (c-half, o-half) blocks alternate between the two PSUM
#     pools so evictions never stall the PE.
#   scoresT[k,q] over d: matmul(lhsT=KT k-tile, rhs=QT q-block); ACT Exp
#     evicts P^T[k,q] bf16 (no max-subtraction: |scores| <~ 6 here, exp is
#     safe in fp32). P stays UNNORMALIZED; 1/rowsum is applied as the
#     per-partition ACT scale of the output projection.
#   attn_outT[dv,q] = matmul(lhsT=V token-major tile, rhs=P^T) over 16
#     k-tiles, DVE-evicted (scalar engine carries zero DMAs and only ACTs,
#     so PSUM evictions never queue behind descriptor generation).
#   out[tok,dout] = matmul(lhsT=attn_outT tile, rhs=Wvo chunk)
# Row sums: incremental DVE adds over k-tiles of P^T, then a GpSimd partition
# reduce + DRAM-bounce partition scatter + reciprocal, all off the PE path.
# V/Wvo DMA issues are gated behind a dummy DVE copy that depends on the
# first q_eff block, so the early HBM bandwidth all goes to A/q/K.

import functools

import ml_dtypes
import numpy as np

B, S, D = 4, 2048, 1024
N_CORES = 8
P = 128
DT = D // P        # 8 d-tiles of 128
TQ = S // 2        # 1024 query rows per core
NQ = TQ // P       # 8 q-tiles
NK = S // P        # 16 k-tiles
SCALE = 1.0 / np.sqrt(np.float32(D))  # 1/32
BF16 = ml_dtypes.bfloat16


@functools.lru_cache(maxsize=1)
def _build():
    import concourse.bass as bass  # noqa: F401  (registers engines)
    import concourse.mybir as mybir
    import concourse.tile as tile
    from concourse import bacc

    f32 = mybir.dt.float32
    bf16 = mybir.dt.bfloat16

    nc = bacc.Bacc("TRN2", target_bir_lowering=False, debug=False,
                   num_devices=N_CORES)

    qT = nc.dram_tensor("qT", [D, TQ], bf16, kind="ExternalInput")
    kT = nc.dram_tensor("kT", [D, S], bf16, kind="ExternalInput")
    vtm = nc.dram_tensor("vtm", [S, D], bf16, kind="ExternalInput")
    wa = nc.dram_tensor("wa", [D, D], bf16, kind="ExternalInput")    # A
    wvo = nc.dram_tensor("wvo", [D, D], bf16, kind="ExternalInput")  # Wv@Wo
    bc = nc.dram_tensor("bc", [D], f32, kind="ExternalInput")  # bq@Wk^T/32
    out_d = nc.dram_tensor("out", [TQ, D], f32, kind="ExternalOutput")

    Ident = mybir.ActivationFunctionType.Identity
    Exp = mybir.ActivationFunctionType.Exp

    with tile.TileContext(nc) as tc:
        with (
            tc.tile_pool(name="const", bufs=1) as const,
            tc.tile_pool(name="wpool", bufs=2) as wpool,
            tc.tile_pool(name="big", bufs=1) as big,
            tc.tile_pool(name="work", bufs=2) as work,
            tc.tile_pool(name="sums", bufs=1) as sums,
            tc.tile_pool(name="dram", bufs=1, space="DRAM") as dram,
            tc.tile_pool(name="mmps", bufs=4, space="PSUM") as mmps,
            tc.tile_pool(name="scps", bufs=4, space="PSUM") as scps,
        ):
            # ---- constants ----
            bc_sb = const.tile([P, DT], f32, tag="bc")
            r_all = const.tile([P, NQ], f32, tag="rall")
            warm = const.tile([P, 640], bf16, tag="warm")

            # ---- persistent intermediates ----
            QT = big.tile([P, DT, TQ], bf16, tag="QT")       # 2 MB  q_eff^T
            KT = big.tile([P, DT, S], bf16, tag="KT")        # 4 MB
            Vtm = big.tile([P, NK, D], bf16, tag="Vtm")      # 4 MB (token-major)
            PT = big.tile([P, NK, TQ], bf16, tag="PT")       # 4 MB  exp(s)^T
            aoT = big.tile([P, DT, TQ], bf16, tag="aoT")     # 2 MB attn_out^T
            qsb = big.tile([P, DT, TQ], bf16, tag="qsb")     # 2 MB raw q^T
            w_a = wpool.tile([P, DT, D], bf16, tag="w", name="w_a")
            w_o = wpool.tile([P, DT, D], bf16, tag="w", name="w_o")

            # ---- PE warm-up: a dozen matmuls on a memset tile keep the
            # PE busy from ~6us (during the input DMA latency) so the HAM
            # clock-gate is already at 8/8 when the real stream starts ----
            nc.gpsimd.memset(warm[:], 0.001)
            warmps = mmps.tile([P, 512], f32, tag="mm", name="warmps")
            for _ in range(6):
                nc.tensor.matmul(warmps[:], warm[:, 512:640], warm[:, 0:512],
                                 start=True, stop=True)

            # ---- input DMAs: contiguous chunks, in consumption order ----
            wa_ap = wa.ap().rearrange("(dt p) n -> p dt n", p=P)
            q_ap = qT.ap().rearrange("(dt p) t -> p dt t", p=P)
            # a throwaway first DMA per queue absorbs the ~2us DMA-ring
            # spin-up latency so the first real chunk lands sooner
            spin = work.tile([P, 64], bf16, tag="spin")
            nc.gpsimd.dma_start(spin[:, 0:32], wa_ap[:, 0, 0:32])
            nc.sync.dma_start(spin[:, 32:64], q_ap[:, 0, 0:32])
            # gpsimd queue: the half of A the very first matmuls need, then
            # the (tiny, partition-major) bias, then the rest of A by
            # full-dt chunks (dt-outer q_eff consumes them in this order)
            nc.gpsimd.dma_start(w_a[:, 0, 0:512], wa_ap[:, 0, 0:512])
            nc.gpsimd.dma_start(bc_sb[:], bc.ap().rearrange("(p o) -> p o", p=P))
            nc.gpsimd.dma_start(w_a[:, 0, 512:1024], wa_ap[:, 0, 512:1024])
            for dt_i in range(1, DT):
                nc.gpsimd.dma_start(w_a[:, dt_i, :], wa_ap[:, dt_i, :])
            # sync queue: q by (c-half, dt) chunks
            for ch in range(2):
                for dt_i in range(DT):
                    sl = slice(ch * 512, (ch + 1) * 512)
                    nc.sync.dma_start(qsb[:, dt_i, sl], q_ap[:, dt_i, sl])
            # gpsimd continues with K (needed when scores start, ~40us in)
            k_ap = kT.ap().rearrange("(dt p) t -> p dt t", p=P)
            for dt_i in range(DT):
                nc.gpsimd.dma_start(KT[:, dt_i, :], k_ap[:, dt_i, :])

            # ---- q_eff: four (c, o-half) blocks, dt-outer accumulation ----
            # QT[:, o, csl] = sum_dt A[:, dt, oP:+P]^T qsb[:, dt, csl] + bc
            def qeff_block(ch, oh, pool, tg):
                csl = slice(ch * 512, (ch + 1) * 512)
                pss = [pool.tile([P, 512], f32, tag=tg,
                                 name=f"qe{ch}{oh}{i}") for i in range(4)]
                for dt_i in range(DT):
                    for oi in range(4):
                        o = oh * 4 + oi
                        nc.tensor.matmul(
                            pss[oi][:],
                            w_a[:, dt_i, o * P:(o + 1) * P],
                            qsb[:, dt_i, csl],
                            start=(dt_i == 0),
                            stop=(dt_i == DT - 1),
                        )
                for oi in range(4):
                    o = oh * 4 + oi
                    nc.scalar.activation(
                        QT[:, o, csl], pss[oi][:],
                        Ident, bias=bc_sb[:, o:o + 1], scale=1.0,
                    )

            qeff_block(0, 0, mmps, "mm")
            qeff_block(0, 1, scps, "sc")
            qeff_block(1, 0, mmps, "mm")
            qeff_block(1, 1, scps, "sc")

            # gpsimd tail: V and Wvo loads, gated on the first q_eff block
            # having evicted, so they don't steal early HBM bandwidth
            gate = work.tile([P, 1], bf16, tag="gate")
            nc.gpsimd.tensor_copy(gate[:], QT[:, 3, 0:1])
            v_ap = vtm.ap().rearrange("(kt p) d -> p kt d", p=P)
            for c in range(4):
                nc.gpsimd.dma_start(Vtm[:, c * 4:(c + 1) * 4, :],
                                    v_ap[:, c * 4:(c + 1) * 4, :])
            wo_ap = wvo.ap().rearrange("(dt p) n -> p dt n", p=P)
            for dt_i in range(DT):
                nc.gpsimd.dma_start(w_o[:, dt_i, :], wo_ap[:, dt_i, :])

            # ---- attention over 512-wide q-blocks ----
            s1_tiles = {}

            def s1_add(blk, kt):
                # incremental k-tile sum on the DVE (contiguous reads)
                qsl = slice(blk * 512, (blk + 1) * 512)
                s1 = s1_tiles[blk]
                if kt == 0:
                    nc.vector.tensor_copy(s1[:], PT[:, 0, qsl])
                else:
                    nc.vector.tensor_tensor(
                        s1[:], PT[:, kt, qsl], s1[:], mybir.AluOpType.add)

            def score_blk(blk):
                qsl = slice(blk * 512, (blk + 1) * 512)
                s1_tiles[blk] = sums.tile([P, 512], f32, tag=f"s1b{blk}",
                                          name=f"s1b{blk}")
                for kt in range(NK):
                    sc = scps.tile([P, 512], f32, tag="sc")
                    for dt_i in range(DT):
                        nc.tensor.matmul(
                            sc[:],
                            KT[:, dt_i, kt * P:(kt + 1) * P],
                            QT[:, dt_i, qsl],
                            start=(dt_i == 0),
                            stop=(dt_i == DT - 1),
                        )
                    nc.scalar.activation(PT[:, kt, qsl], sc[:], Exp)
                    s1_add(blk, kt)

            def row_sums(blk):
                # 1/rowsum for the block's 512 q positions -> r_all[:, 4b:4b+4]
                s1 = s1_tiles[blk]
                from concourse import bass_isa
                s2 = sums.tile([P, 512], f32, tag=f"s2b{blk}", name=f"s2b{blk}")
                nc.gpsimd.partition_all_reduce(
                    s2[:], s1[:], channels=P, reduce_op=bass_isa.ReduceOp.add)
                rsc = work.tile([P, 4], f32, tag="rsc")
                # partition-scatter must bounce through DRAM: SBUF partition
                # dims are physical and cannot be built from free strides
                rd = dram.tile([1, 512], f32, name=f"rd{blk}")
                nc.sync.dma_start(rd[:], s2[0:1, :])
                nc.sync.dma_start(
                    rsc[:], rd[:].rearrange("a (j p) -> p (a j)", p=P))
                nc.vector.reciprocal(r_all[:, blk * 4:(blk + 1) * 4], rsc[:])

            def attn_v(blk):
                # aoT[:, dvt, qsl] = sum_kt Vtm[:, kt, dvt*P:+P]^T P^T[:, kt, qsl]
                qsl = slice(blk * 512, (blk + 1) * 512)
                for dvt in range(DT):
                    av = mmps.tile([P, 512], f32, tag="mm")
                    for kt in range(NK):
                        nc.tensor.matmul(
                            av[:],
                            Vtm[:, kt, dvt * P:(dvt + 1) * P],
                            PT[:, kt, qsl],
                            start=(kt == 0),
                            stop=(kt == NK - 1),
                        )
                    nc.vector.tensor_copy(aoT[:, dvt, qsl], av[:])

            def out_proj(tt, width=512):
                # out[tok, dout], scaled by 1/rowsum (tokens on partitions).
                # The very last tile runs at width 256 so its trailing
                # ACT+DMA chain (the kernel's tail) is half as long.
                fin = work.tile([P, D], f32, tag="fin")
                for dc in range(D // width):
                    osl = slice(dc * width, (dc + 1) * width)
                    ps = scps.tile([P, 512], f32, tag="sc")
                    for dvt in range(DT):
                        nc.tensor.matmul(
                            ps[:, 0:width],
                            aoT[:, dvt, tt * P:(tt + 1) * P],
                            w_o[:, dvt, osl],
                            start=(dvt == 0),
                            stop=(dvt == DT - 1),
                        )
                    nc.scalar.activation(
                        fin[:, osl], ps[:, 0:width],
                        Ident, scale=r_all[:, tt:tt + 1],
                    )
                    nc.sync.dma_start(
                        out_d.ap()[tt * P:(tt + 1) * P, osl], fin[:, osl])

            score_blk(0)
            attn_v(0)
            row_sums(0)
            score_blk(1)
            attn_v(1)
            row_sums(1)
            for tt in range(NQ):
                out_proj(tt, width=(256 if tt == NQ - 1 else 512))

    nc.compile()
    return nc


def _numpy_reference(query, key, value, mask, Wq, bq, Wk, bk, Wv, bv, Wo, bo):
    q = query @ Wq + bq
    k = key @ Wk + bk
    v = value @ Wv + bv
    s = np.einsum("bsd,btd->bst", q, k) / np.sqrt(np.float32(q.shape[-1]))
    s = np.where(mask == 0, np.float32(-1e9), s)
    s = s - s.max(axis=-1, keepdims=True)
    e = np.exp(s)
    p = e / e.sum(axis=-1, keepdims=True)
    x = np.einsum("bst,btd->bsd", p, v)
    return (x @ Wo + bo).astype(np.float32)


def kernel(query, key, value, mask, Wq, bq, Wk, bk, Wv, bv, Wo, bo):
    query = np.asarray(query, np.float32)
    key = np.asarray(key, np.float32)
    value = np.asarray(value, np.float32)
    mask = np.asarray(mask)
    if not np.all(mask != 0):
        # This problem's mask is always all-ones; keep a correct fallback.
        return _numpy_reference(query, key, value, mask, Wq, bq, Wk, bk,
                                Wv, bv, Wo, bo)

    from concourse.bass_utils import run_bass_kernel_spmd

    nc = _build()

    Wq32 = np.asarray(Wq, np.float32)
    Wk32 = np.asarray(Wk, np.float32)
    Wv32 = np.asarray(Wv, np.float32)
    Wo32 = np.asarray(Wo, np.float32)
    A = (Wq32 @ Wk32.T * SCALE).astype(BF16)          # scores = (q A + c) k^T
    Wvo = (Wv32 @ Wo32).astype(BF16)
    c_vec = (np.asarray(bq, np.float32) @ Wk32.T * SCALE).astype(np.float32)
    # pack partition-major: kernel reads bc_sb[p, o] = c_vec[o*P + p]
    c_vec = np.ascontiguousarray(c_vec.reshape(DT, P).T).ravel()
    bo_eff = (np.asarray(bv, np.float32) @ Wo32
              + np.asarray(bo, np.float32)).astype(np.float32)

    kT_b = [np.ascontiguousarray(key[b].T).astype(BF16) for b in range(B)]
    v_b = [np.ascontiguousarray(value[b]).astype(BF16) for b in range(B)]

    in_maps = []
    for cidx in range(N_CORES):
        b, h = divmod(cidx, 2)
        sl = slice(h * TQ, (h + 1) * TQ)
        in_maps.append({
            "qT": np.ascontiguousarray(query[b, sl].T).astype(BF16),
            "kT": kT_b[b],
            "vtm": v_b[b],
            "wa": A, "wvo": Wvo, "bc": c_vec,
        })

    global _last_in_maps
    _last_in_maps = in_maps
    res = run_bass_kernel_spmd(nc, in_maps, list(range(N_CORES)))

    out = np.empty((B, S, D), np.float32)
    for cidx in range(N_CORES):
        b, h = divmod(cidx, 2)
        out[b, h * TQ:(h + 1) * TQ] = res.results[cidx]["out"]
    out += bo_eff
    return out
